# revision 2
# baseline (speedup 1.0000x reference)
"""AtrousFourWayMamba Trainium2 kernel (8-core SPMD, d_inner-sharded). v3.

Self-contained: hardcodes all shapes. Accepts FULL inputs, returns FULL output.

Structure per core (128 of 1024 d_inner channels):
- head: in_proj x-GEMM (z-half deferred), per-branch causal conv on PE
  (diagonal weights) + SiLU, x_proj partials, one AllReduce per branch
  (C, B, A order; a tiny warmup AR absorbs collective launch overhead)
- selective scans as PAIRED states: one DVE tensor_tensor_scan covers two
  states (4000 cols) with a zero-decay boundary column resetting h between
  them; B/C rows broadcast as 2-state pair DMAs rotated over the sync/
  scalar/gpsimd queues with a 3-deep ring (no per-state stalls); da on ACT
  (exp with per-partition A scale); dbu multiplied in place into the bb
  ring slot; g = h*C in place in the scan output; y accumulated over states
  on PE via identity matmuls into PSUM, D*u folded in via diagonal weights
- silu(z) precomputed into each branch's finalize tile so finalize is one
  in-place PSUM multiply; branch un-permutes run inside the scan phase
- BiAttn: LN stats (bf16) ride inside the AllToAll payload as extra rows
  (282-row blocks = 250 ori rows + 32 stats rows), eliminating a separate
  stats AllReduce; gate algebra host-folded (glw = ln_g*grw, negs2w,
  ln_b term folded into the post-AR bias) so the AR3 chain is short;
  att_out and out_proj fused into one attn-scaled GEMM; direct out_shard
  stores (host concatenates shards and adds the output bias)
"""
import os
import sys
import types
import ctypes
import contextlib
from contextlib import ExitStack

sys.path.insert(0, '/opt/trn_rl_repo')

import numpy as np


def _install_axon_hooks_shim():
    try:
        from antenv.axon_hooks import get_axon_ntff_profile_hook  # noqa
        return
    except ImportError:
        pass
    so_path = "/opt/axon/libaxon_pjrt.so"
    hook = None
    if os.path.exists(so_path):
        lib = ctypes.CDLL(so_path)
        if hasattr(lib, "axon_start_nrt_profile"):
            lib.axon_start_nrt_profile.argtypes = [ctypes.POINTER(ctypes.c_int64), ctypes.c_size_t]
            lib.axon_start_nrt_profile.restype = ctypes.c_int64
            lib.axon_stop_nrt_profile.argtypes = [ctypes.c_char_p]
            lib.axon_stop_nrt_profile.restype = ctypes.c_int64

            @contextlib.contextmanager
            def _hook(output_dir, device_ids):
                import jax
                jax.devices()
                if device_ids:
                    ids = (ctypes.c_int64 * len(device_ids))(*device_ids)
                    rc = lib.axon_start_nrt_profile(ids, len(device_ids))
                else:
                    rc = lib.axon_start_nrt_profile(None, 0)
                if rc != 0:
                    raise RuntimeError(f"axon_start_nrt_profile rc={rc}")
                try:
                    yield
                finally:
                    n = lib.axon_stop_nrt_profile(str(output_dir).encode())
                    print(f"profile: {n} file(s) written to {output_dir}", file=sys.stderr)

            hook = _hook
    import antenv
    mod = types.ModuleType("antenv.axon_hooks")
    mod.get_axon_ntff_profile_hook = lambda: hook
    mod.set_axon_ntff_profile_hook = lambda h: None
    sys.modules["antenv.axon_hooks"] = mod
    antenv.axon_hooks = mod


_install_axon_hooks_shim()

import concourse.bass as bass
import concourse.bacc as bacc
import concourse.tile as tile
from concourse import mybir
from concourse.bass_utils import run_bass_kernel_spmd

F32 = mybir.dt.float32
BF16 = mybir.dt.bfloat16
FP16 = mybir.dt.float16
AF = mybir.ActivationFunctionType
OP = mybir.AluOpType
AX = mybir.AxisListType

NC_ = 8
DM = 512
DI = 1024
DL = DI // NC_     # 128
NS = 16
RK = 32
L = 2000
CH = 500
NQ = L // CH
HL = L // 2        # 1000

ACCUM_G = False    # accum_op=mult rejected by BIR verifier; cc ring + DVE mult
POOL_G = False     # g-mult on gpsimd (Pool) instead of DVE
POOL_DBU = ()      # pair indices (global 0..23) whose dbu runs on Pool

_CACHE = {}


def _seq_views(x, scan, pre=3):
    v = x[:, pre:].rearrange("p (h w) -> p h w", w=10)
    if scan == 0:
        return [
            (slice(0, 500), v[:, 0::2, 0::2]),
            (slice(500, 1000), v[:, 1::2, 0::2].rearrange("p h w -> p w h")),
            (slice(1000, 1500), v[:, 198::-2, 9::-2]),
            (slice(1500, 2000), v[:, 199::-2, 9::-2].rearrange("p h w -> p w h")),
        ]
    elif scan == 1:
        return [
            (slice(0, 1000), v[:, 0::2, :]),
            (slice(1000, 2000), v[:, 1::2, :]),
        ]
    raise ValueError(scan)


def _r3(ap2d, a, b):
    return ap2d.rearrange("p (a b) -> p a b", b=b)


def _build(dbg=False):
    nc = bacc.Bacc("TRN2", target_bir_lowering=False, debug=False, num_devices=NC_)

    def dump(name, ap):
        if not dbg:
            return
        d = nc.dram_tensor(f"dbg_{name}", list(ap.shape), ap.dtype,
                           kind="ExternalOutput").ap()
        nc.sync.dma_start(d, ap)

    def din(name, shape, dt=F32):
        return nc.dram_tensor(name, list(shape), dt, kind="ExternalInput").ap()

    io = dict(
        hidT=din("hidT", (DM, L), BF16),
        wxT=din("wxT", (DM, DL), BF16),
        wzT=din("wzT", (DM, DL), BF16),
        ones_colf=din("ones_colf", (128, 1)),
        identT=din("identT", (128, 128), BF16),
        glw=din("glw", (DL, 512), BF16),
        negs2w=din("negs2w", (1, 512)),
        grbT=din("grbT", (128, 4)),
        cswT=din("cswT", (512, DI), BF16),
        wcombF=din("wcombF", (DI, DM), BF16),
        csbF=din("csbF", (128, 8)),
    )
    for s in range(3):
        io[f"convw{s}"] = din(f"convw{s}", (4 * DL, DL), BF16)
        io[f"convb{s}"] = din(f"convb{s}", (DL, 1))
        io[f"xwT{s}"] = din(f"xwT{s}", (DL, 64), BF16)
        io[f"dtwT{s}"] = din(f"dtwT{s}", (RK, DL), BF16)
        io[f"dtb{s}"] = din(f"dtb{s}", (DL, 1))
        io[f"avec{s}"] = din(f"avec{s}", (DL, NS))
        io[f"ddiag{s}"] = din(f"ddiag{s}", (DL, DL), BF16)
    out_shard = nc.dram_tensor("out_shard", [L // NC_, DM], BF16, kind="ExternalOutput").ap()

    with tile.TileContext(nc) as tc, ExitStack() as ctx:
        cons = ctx.enter_context(tc.tile_pool(name="cons", bufs=1))
        big = ctx.enter_context(tc.tile_pool(name="big", bufs=1))
        work = ctx.enter_context(tc.tile_pool(name="work", bufs=2))
        psum = ctx.enter_context(tc.tile_pool(name="psum", bufs=2, space="PSUM"))
        dram = ctx.enter_context(tc.tile_pool(name="dram", bufs=1, space="DRAM"))

        ar0_in = dram.tile([1, 8], F32)
        ar0_out = dram.tile([1, 8], F32, addr_space="Shared")
        warm = cons.tile([1, 8], F32, name="warm")
        nc.vector.memset(warm[:], 1.0)
        nc.sync.dma_start(ar0_in[:], warm[:])
        nc.gpsimd.collective_compute(
            "AllReduce", OP.add, replica_groups=[list(range(NC_))],
            ins=[ar0_in.opt()], outs=[ar0_out.opt()])

        # ---------- urgent loads (sync + scalar) ----------
        wxT_sb = [cons.tile([128, DL], BF16, name=f"wxT_sb{k}") for k in range(4)]
        wzT_sb = [cons.tile([128, DL], BF16, name=f"wzT_sb{k}") for k in range(4)]
        for k in range(4):
            (nc.sync if k % 2 == 0 else nc.scalar).dma_start(
                wxT_sb[k][:], io["wxT"][128 * k:128 * (k + 1), :])
            (nc.sync if k % 2 == 0 else nc.scalar).dma_start(
                wzT_sb[k][:], io["wzT"][128 * k:128 * (k + 1), :])
        identT = cons.tile([128, 128], BF16, name="identT")
        nc.sync.dma_start(identT[:], io["identT"])
        hid3 = [cons.tile([128, CH], BF16, name=f"hid3_{q}") for q in range(NQ)]
        for q in range(NQ):
            nc.gpsimd.dma_start(hid3[q][:], io["hidT"][384:512, q * CH:(q + 1) * CH])

        # ---------- deferred consts on gpsimd queue (branch C first) ----------
        convw, convb, xwT, dtwT, dtb, avec, ddiag = {}, {}, {}, {}, {}, {}, {}
        for s in [2, 1, 0]:
            cw4 = [cons.tile([DL, DL], BF16, name=f"convw{s}_{k}") for k in range(4)]
            for k in range(4):
                nc.gpsimd.dma_start(cw4[k][:], io[f"convw{s}"][128 * k:128 * (k + 1), :])
            convw[s] = cw4
            convb[s] = cons.tile([DL, 1], F32, name=f"convb{s}")
            nc.gpsimd.dma_start(convb[s][:], io[f"convb{s}"])
            xwT[s] = cons.tile([DL, 64], BF16, name=f"xwT{s}")
            nc.gpsimd.dma_start(xwT[s][:], io[f"xwT{s}"])
            dtwT[s] = cons.tile([RK, DL], BF16, name=f"dtwT{s}")
            nc.gpsimd.dma_start(dtwT[s][:], io[f"dtwT{s}"])
            dtb[s] = cons.tile([DL, 1], F32, name=f"dtb{s}")
            nc.gpsimd.dma_start(dtb[s][:], io[f"dtb{s}"])
            avec[s] = cons.tile([DL, NS], F32, name=f"avec{s}")
            nc.gpsimd.dma_start(avec[s][:], io[f"avec{s}"])
        for s in range(3):
            ddiag[s] = cons.tile([DL, DL], BF16, name=f"ddiag{s}")
            nc.gpsimd.dma_start(ddiag[s][:], io[f"ddiag{s}"])
        glw_sb = cons.tile([DL, 512], BF16, name="glw_sb")
        nc.gpsimd.dma_start(glw_sb[:], io["glw"])
        negs2w_sb = cons.tile([1, 512], F32, name="negs2w_sb")
        nc.gpsimd.dma_start(negs2w_sb[:], io["negs2w"])
        grbT_sb = cons.tile([128, 4], F32, name="grbT_sb")
        nc.gpsimd.dma_start(grbT_sb[:], io["grbT"])
        csbF_sb = cons.tile([128, 8], F32, name="csbF_sb")
        nc.gpsimd.dma_start(csbF_sb[:], io["csbF"])
        ones_colf = cons.tile([128, 1], F32, name="ones_colf")
        nc.gpsimd.dma_start(ones_colf[:], io["ones_colf"])
        cswT_sb = [cons.tile([128, DI], BF16, name=f"cswT_sb{k}") for k in range(4)]
        for k in range(4):
            nc.gpsimd.dma_start(cswT_sb[k][:], io["cswT"][128 * k:128 * (k + 1), :])
        wcombF = [cons.tile([128, DM], BF16, name=f"wcombF{j}") for j in range(8)]
        for j in range(8):
            nc.gpsimd.dma_start(wcombF[j][:], io["wcombF"][128 * j:128 * (j + 1), :])

        # ---------- collective buffers ----------
        ar1_ins = {s: dram.tile([64, L], BF16, name=f"ar1_in{s}") for s in (0, 1, 2)}
        ar1_outs = {s: dram.tile([64, L], BF16, addr_space="Shared", name=f"ar1_out{s}")
                    for s in (0, 1, 2)}
        ar3_in = dram.tile([1, 512], F32)
        ar3_out = dram.tile([1, 512], F32, addr_space="Shared")
        BLK = 282  # 250 ori rows + 32 stats rows per peer block
        a2a_in = dram.tile([NC_ * BLK, DL], BF16)
        a2a_out = dram.tile([NC_ * BLK, DL], BF16)

        # ================= stage 1: xz GEMM =================
        x_c = big.tile([DL, L + 3], BF16)
        z_c = big.tile([DL, L], BF16)
        nc.vector.memset(x_c[:, 0:3], 0.0)
        for q in range(NQ):
            lsl = slice(q * CH, (q + 1) * CH)
            mmx = psum.tile([128, CH], F32, tag="mm", name=f"mmx{q}")
            for k in range(4):
                if k == 3:
                    hidt = hid3[q]
                else:
                    hidt = work.tile([128, CH], BF16, tag="hidt", bufs=3, name=f"hidt{q}_{k}")
                    nc.sync.dma_start(hidt[0:64, :], io["hidT"][128 * k:128 * k + 64, lsl])
                    nc.scalar.dma_start(hidt[64:128, :], io["hidT"][128 * k + 64:128 * (k + 1), lsl])
                nc.tensor.matmul(mmx[:], wxT_sb[k][:], hidt[:], start=(k == 0), stop=(k == 3))
            nc.scalar.copy(x_c[:, 3 + q * CH:3 + (q + 1) * CH], mmx[:])
        dump("x_c", x_c[:, 3:])

        def z_gemm():
            # deferred z-half of in_proj: runs during branch-C's scans when
            # PE has slack (hidT k<3 re-streamed on the gpsimd queue)
            for q in range(NQ):
                lsl = slice(q * CH, (q + 1) * CH)
                mmz = psum.tile([128, CH], F32, tag="mmc", name=f"mmz{q}")
                for k in range(4):
                    if k == 3:
                        hidt = hid3[q]
                    else:
                        hidt = work.tile([128, CH], BF16, tag="hidt", bufs=3,
                                         name=f"hidt2_{q}_{k}")
                        nc.gpsimd.dma_start(hidt[:], io["hidT"][128 * k:128 * (k + 1), lsl])
                    nc.tensor.matmul(mmz[:], wzT_sb[k][:], hidt[:], start=(k == 0), stop=(k == 3))
                nc.scalar.copy(z_c[:, lsl], mmz[:])
            dump("z_c", z_c[:])

        # ================= stage 2: per-branch conv/silu/x_proj + AR ========
        us = {}
        xps = {}
        for s in [2, 1, 0]:
            if s < 2:
                xp = big.tile([DL, L + 3], BF16, tag="xp", bufs=1, name=f"xp{s}")
                nc.vector.memset(xp[:, 0:3], 0.0)
                for dsl, view in _seq_views(x_c, s):
                    a, b = view.shape[1], view.shape[2]
                    nc.vector.tensor_copy(_r3(xp[:, 3 + dsl.start:3 + dsl.stop], a, b), view)
            else:
                xp = x_c
            xps[s] = xp
            u = big.tile([DL, L], BF16, name=f"u{s}")
            for q in range(NQ):
                lsl = slice(q * CH, (q + 1) * CH)
                mmc = psum.tile([128, CH], F32, tag="mmc", name=f"mmc{s}_{q}")
                for k in range(4):
                    nc.tensor.matmul(mmc[:], convw[s][k][:],
                                     xp[:, q * CH + k:q * CH + k + CH],
                                     start=(k == 0), stop=(k == 3))
                nc.scalar.activation(u[:, lsl], mmc[:], AF.Silu, bias=convb[s][:])
            us[s] = u
            dump(f"u{s}", u[:])
            for q in range(NQ):
                lsl = slice(q * CH, (q + 1) * CH)
                mm = psum.tile([64, CH], F32, tag="mm", name=f"mmxp{s}_{q}")
                nc.tensor.matmul(mm[:], xwT[s][:], u[:, lsl], start=True, stop=True)
                st = work.tile([64, CH], BF16, tag="xdst", bufs=2, name=f"xdst{s}_{q}")
                nc.vector.tensor_copy(st[:], mm[:])
                (nc.scalar if q % 2 == 0 else nc.sync).dma_start(ar1_ins[s][:, lsl], st[:])
            nc.gpsimd.collective_compute(
                "AllReduce", OP.add, replica_groups=[list(range(NC_))],
                ins=[ar1_ins[s].opt()], outs=[ar1_outs[s].opt()])

        # silu(z) for all three branches in the head (Silu table is already
        # loaded): directly into each branch's finalize target tile
        sz_dst = {}
        # then preload the Exp table during the head's idle ACT window
        exp_warm = work.tile([1, 1], F32, tag="expw", bufs=1)

        # ================= scan-phase shared tiles =================
        # rings (manual, explicit): pair layouts [128, 4000]
        bb2r = [big.tile([DL, 2 * L], BF16, name=f"bb2r{i}") for i in range(3)]
        da2r = [big.tile([DL, 2 * L], FP16, name=f"da2r{i}") for i in range(2)]
        for t in da2r:
            nc.vector.memset(t[:, L:L + 1], 0.0)  # zero-decay state boundary
        g2r = [big.tile([DL, 2 * L], BF16, name=f"g2r{i}") for i in range(3)]
        if not ACCUM_G:
            cc2r = [big.tile([DL, 2 * L], BF16, name=f"cc2r{i}") for i in range(2)]

        # per-branch delta / du2 (bufs=2: current + next)
        delta_t = [big.tile([DL, L], BF16, name=f"delta_t{i}") for i in range(2)]
        du2_t = [big.tile([DL, 2 * L], BF16, name=f"du2_t{i}") for i in range(2)]
        stage_t = {s: cons.tile([RK, L], BF16, name=f"stage{s}") for s in (0, 1, 2)}
        sz_t = big.tile([DL, L], BF16, name="sz_t")
        y_t = [big.tile([DL, L], BF16, name=f"y_t{i}") for i in range(2)]
        scratch = big.tile([DL, L], BF16, name="scratch")
        ori = big.tile([DL, L], BF16)
        sz_dst.update({2: y_t[0], 1: y_t[1], 0: sz_t})

        def sz_all():
            for s_ in (2, 1, 0):
                tgt = sz_dst[s_]
                if s_ < 2:
                    for dsl, view in _seq_views(z_c, s_, pre=0):
                        a, b = view.shape[1], view.shape[2]
                        nc.scalar.activation(_r3(tgt[:, dsl], a, b), view, AF.Silu)
                else:
                    nc.scalar.activation(tgt[:], z_c[:], AF.Silu)
        nc.scalar.activation(exp_warm[:], warm[0:1, 0:1], AF.Exp)

        def phase_a(s, slot):
            """stage DMA + dt GEMM + softplus -> delta[slot]; du2[slot]."""
            delta, du2 = delta_t[slot], du2_t[slot]
            nc.scalar.dma_start(stage_t[s][:], ar1_outs[s][0:RK, :])
            mms = []
            for q in range(NQ):
                lsl = slice(q * CH, (q + 1) * CH)
                mm = psum.tile([128, CH], F32, tag="mm", name=f"mmdt{s}_{q}")
                nc.tensor.matmul(mm[:], dtwT[s][:], stage_t[s][:, lsl], start=True, stop=True)
                mms.append((lsl, mm))
            # batched: all Exp then all Ln (2 table switches per branch)
            for lsl, mm in mms:
                nc.scalar.activation(delta[:, lsl], mm[:], AF.Exp, bias=dtb[s][:])
            for lsl, mm in mms:
                nc.scalar.activation(delta[:, lsl], delta[:, lsl], AF.Ln, bias=1.0)
            for off in (0, L):
                for h in range(2):
                    hsl = slice(h * HL, (h + 1) * HL)
                    nc.vector.tensor_tensor(du2[:, off + h * HL:off + (h + 1) * HL],
                                            delta[:, hsl], us[s][:, hsl], OP.mult)
            dump(f"delta{s}", delta[:])
            return delta, du2

        ypsum = psum.tile([128, 2048], F32, tag="y", bufs=1, name="ypsum")
        ys = {}

        def finalize(s, _unused):
            """y = ypsum * silu(z) in place over the head-precomputed sz."""
            y = sz_dst[s]
            yp_view = ypsum.rearrange("p (a b) -> p a b", b=512)[:, :, 0:CH]
            nc.vector.tensor_tensor(_r3(y[:], NQ, CH), yp_view,
                                    _r3(y[:], NQ, CH), OP.mult)
            dump(f"ys{s}", y[:])
            return y

        # ---- unified scan loop: branches C(2), B(1), A(0), paired states ----
        qs = [nc.sync, nc.scalar, nc.gpsimd]
        delta, du2 = phase_a(2, 0)
        nxt = {2: 1, 1: 0}
        slots = {2: 0, 1: 1, 0: 0}
        pend = {}
        yout = {}
        t_glob = 0
        for s in [2, 1, 0]:
            ybank_started = [False] * 4
            for k in range(8):
                n0 = 2 * k
                bb = bb2r[t_glob % 3]
                da2 = da2r[t_glob % 2]
                g2 = g2r[t_glob % 3]
                cc = cc2r[t_glob % 2]
                bsrc = bass.AP(ar1_outs[s].tensor, (RK + n0) * L, [[0, 128], [L, 2], [1, L]])
                csrc = bass.AP(ar1_outs[s].tensor, (RK + NS + n0) * L,
                               [[0, 128], [L, 2], [1, L]])
                if t_glob < 2:
                    # gpsimd queue is blocked behind collective triggers early
                    # on; keep the first pairs on the HW queues
                    nc.scalar.dma_start(bb[:], bsrc)
                    nc.sync.dma_start(cc[:], csrc)
                else:
                    qs[t_glob % 3].dma_start(bb[:], bsrc)
                    qs[(t_glob + 1) % 3].dma_start(cc[:], csrc)
                nc.scalar.activation(da2[:, 0:L], delta[:], AF.Exp,
                                     scale=avec[s][:, n0:n0 + 1])
                nc.scalar.activation(da2[:, L + 1:2 * L], delta[:, 1:L], AF.Exp,
                                     scale=avec[s][:, n0 + 1:n0 + 2])
                nc.vector.tensor_tensor(bb[:], du2[:], bb[:], OP.mult)  # dbu in place
                nc.vector.tensor_tensor_scan(g2[:], da2[:], bb[:], 0.0, OP.mult, OP.add)
                if s == 2 and k == 0:
                    dump("h00", g2[:, 0:L])
                nc.vector.tensor_tensor(g2[:], g2[:], cc[:], OP.mult)
                for j in range(2):
                    for q in range(NQ):
                        gsl = slice(j * L + q * CH, j * L + q * CH + CH)
                        nc.tensor.matmul(ypsum[:, 512 * q:512 * q + CH],
                                         identT[:], g2[:, gsl],
                                         start=not ybank_started[q], stop=False)
                        ybank_started[q] = True
                if s == 2 and k == 1:
                    z_gemm()
                if s == 2 and k == 4:
                    sz_all()
                if k == 5 and s in nxt:
                    sn = nxt[s]
                    pend[sn] = phase_a(sn, slots[sn])
                t_glob += 1
            for q in range(NQ):
                lsl = slice(q * CH, (q + 1) * CH)
                nc.tensor.matmul(ypsum[:, 512 * q:512 * q + CH], ddiag[s][:], us[s][:, lsl],
                                 start=False, stop=True)
            yout[s] = finalize(s, 0)
            if s == 1:
                # scratch = row-interleave(yB) + yC: runs during branch A's
                # scans; tail then only adds yA's un-permuted quadrants
                yC, yB = yout[2], yout[1]
                vT = scratch.rearrange("p (h w) -> p h w", w=10)
                vyC = yC.rearrange("p (h w) -> p h w", w=10)
                for p_ in range(2):
                    nc.vector.tensor_tensor(
                        vT[:, p_::2, :], _r3(yB[:, 1000 * p_:1000 * (p_ + 1)], 100, 10),
                        vyC[:, p_::2, :], OP.add)
            if s in nxt:
                delta, du2 = pend[nxt[s]]
        yA = yout[0]
        vC = ori.rearrange("p (h w) -> p h w", w=10)
        vT = scratch.rearrange("p (h w) -> p h w", w=10)
        q00 = _r3(yA[:, 0:500], 100, 5)
        q10 = yA[:, 500:1000].rearrange("p (w h) -> p h w", h=100)
        q01 = yA[:, 1000:1500][:, ::-1].rearrange("p (h w) -> p h w", w=5)
        q11 = yA[:, 1500:2000][:, ::-1].rearrange("p (w h) -> p h w", h=100)
        nc.vector.tensor_tensor(vC[:, 0::2, 0::2], vT[:, 0::2, 0::2], q00, OP.add)
        nc.vector.tensor_tensor(vC[:, 1::2, 0::2], vT[:, 1::2, 0::2], q10, OP.add)
        nc.vector.tensor_tensor(vC[:, 0::2, 1::2], vT[:, 0::2, 1::2], q01, OP.add)
        nc.vector.tensor_tensor(vC[:, 1::2, 1::2], vT[:, 1::2, 1::2], q11, OP.add)
        dump("ori", ori[:])

        # preload the Sqrt table while the last scan drains
        sq_warm = work.tile([1, 1], F32, tag="sqw", bufs=1)
        nc.scalar.activation(sq_warm[:], warm[0:1, 0:1], AF.Sqrt)

        # ================= stage 5: BiAttn =================
        LC = 125
        oriT = big.tile([128, 16 * 128], BF16)
        s1T = cons.tile([128, 16], BF16)
        s2T = cons.tile([128, 16], BF16)
        nc.vector.memset(s1T[:], 0.0)
        nc.vector.memset(s2T[:], 0.0)
        for gidx in range(4):
            otp = psum.tile([LC, 512], BF16, tag="mm", name=f"otp{gidx}")
            for j in range(4):
                lc = gidx * 4 + j
                csl = slice(LC * lc, LC * (lc + 1))
                nc.tensor.transpose(otp[:, 128 * j:128 * (j + 1)], ori[:, csl], identT[:])
            osl = slice(512 * gidx, 512 * (gidx + 1))
            nc.scalar.copy(oriT[0:LC, osl], otp[:])
            for jj in range(2):
                a2a_dst = bass.AP(a2a_in.tensor, BLK * (2 * gidx + jj) * DL,
                                  [[DL, LC], [LC * DL, 2], [1, DL]])
                o2 = oriT[0:LC, 512 * gidx + 256 * jj:512 * gidx + 256 * (jj + 1)]
                (nc.sync if gidx % 2 == 0 else nc.scalar).dma_start(
                    a2a_dst, _r3(o2, 2, DL))
            s1o = s1T[0:LC, 4 * gidx:4 * gidx + 4].rearrange("p (a b) -> p a b", b=1)
            with nc.allow_low_precision(reason="stats ride the A2A as bf16; summed in f32 after"):
                nc.vector.tensor_reduce(s1o, _r3(otp[:], 4, 128), axis=AX.X, op=OP.add)
            sq = work.tile([LC, 512], BF16, tag="sq", bufs=1, name=f"sq{gidx}")
            nc.vector.tensor_tensor(sq[:], oriT[0:LC, osl], oriT[0:LC, osl], OP.mult)
            s2o = s2T[0:LC, 4 * gidx:4 * gidx + 4].rearrange("p (a b) -> p a b", b=1)
            with nc.allow_low_precision(reason="stats ride the A2A as bf16; summed in f32 after"):
                nc.vector.tensor_reduce(s2o, _r3(sq[:], 4, 128), axis=AX.X, op=OP.add)
        st_scr = dram.tile([1, 4096], BF16, name="st_scr")
        nc.sync.dma_start(bass.AP(st_scr.tensor, 0, [[1, 1], [1, 2048]]), s1T[:])
        nc.scalar.dma_start(bass.AP(st_scr.tensor, 2048, [[1, 1], [1, 2048]]), s2T[:])
        st_rep = bass.AP(a2a_in.tensor, 250 * DL, [[BLK * DL, NC_], [1, 4096]])
        nc.sync.dma_start(st_rep, bass.AP(st_scr.tensor, 0, [[0, NC_], [1, 4096]]))
        nc.gpsimd.collective_compute(
            "AllToAll", OP.bypass, replica_groups=[list(range(NC_))],
            ins=[a2a_in.opt()], outs=[a2a_out.opt()])
        rT = []
        rqs = [nc.sync, nc.scalar, nc.gpsimd]
        stparts = []
        for p in range(8):
            stp = work.tile([128, 32], BF16, tag="stp", bufs=8, name=f"stp{p}")
            rqs[p % 3].dma_start(
                stp[:], bass.AP(a2a_out.tensor, (BLK * p + 250) * DL,
                                [[16, 128], [2048, 2], [1, 16]]))
            stparts.append(stp)
        for p in range(8):
            for h in range(2):
                rb = work.tile([LC, DL], BF16, tag="rb", bufs=6, name=f"rb{p}_{h}")
                rqs[(2 * p + h) % 3].dma_start(
                    rb[:], a2a_out[BLK * p + LC * h:BLK * p + LC * h + LC, :])
                rtp = psum.tile([DL, LC], BF16, tag="mmc", name=f"rtp{p}_{h}")
                nc.tensor.transpose(rtp[:], rb[:], identT[0:LC, 0:LC])
                rt = work.tile([DL, LC], BF16, tag="rt", bufs=16, name=f"rt{p}_{h}")
                if (2 * p + h) % 2 == 0:
                    nc.vector.tensor_copy(rt[:], rtp[:])
                else:
                    nc.scalar.copy(rt[:], rtp[:])
                rT.append(rt)
        stT = cons.tile([128, 32], F32)
        nc.vector.tensor_tensor(stT[:], stparts[0][:], stparts[1][:], OP.add)
        for p in range(2, 8):
            nc.vector.tensor_tensor(stT[:], stT[:], stparts[p][:], OP.add)
        muT = cons.tile([128, 16], F32)
        varT = cons.tile([128, 16], F32)
        rstdT = cons.tile([128, 16], BF16)
        rstdTf = cons.tile([128, 16], F32)
        murT = cons.tile([128, 16], F32)
        nc.scalar.mul(muT[:], stT[:, 0:16], 1.0 / DI)
        nc.scalar.mul(varT[:], stT[:, 16:32], 1.0 / DI)
        tmp16 = cons.tile([128, 16], F32)
        nc.vector.tensor_tensor(tmp16[:], muT[:], muT[:], OP.mult)
        nc.vector.tensor_tensor(varT[:], varT[:], tmp16[:], OP.subtract)
        eps = work.tile([128, 1], F32, tag="eps", bufs=1)
        nc.vector.memset(eps[:], 1e-5)
        nc.scalar.activation(rstdTf[:], varT[:], AF.Sqrt, bias=eps[:])
        nc.vector.reciprocal(rstdTf[:], rstdTf[:])
        nc.vector.tensor_copy(rstdT[:], rstdTf[:])
        nc.vector.tensor_tensor(murT[:], muT[:], rstdTf[:], OP.mult)
        murX = work.tile([128, 1], F32, tag="murX", bufs=1)
        nc.vector.reduce_sum(murX[:], murT[:], axis=AX.X)
        mmsm = psum.tile([1, 1], F32, tag="mm", name="mmsm")
        nc.tensor.matmul(mmsm[:], murX[:], ones_colf[:], start=True, stop=True)
        smur = work.tile([1, 1], F32, tag="smur", bufs=1)
        nc.vector.tensor_copy(smur[:], mmsm[:])
        s1dp = psum.tile([DL, 1], F32, tag="mm", name="s1dp")
        for lc in range(16):
            osl = slice(128 * lc, 128 * lc + 128)
            nc.tensor.matmul(s1dp[:], oriT[0:LC, osl], rstdT[0:LC, lc:lc + 1],
                             start=(lc == 0), stop=(lc == 15))
        s1ds = work.tile([DL, 1], BF16, tag="gd", bufs=1)
        nc.vector.tensor_copy(s1ds[:], s1dp[:])
        mmg = psum.tile([1, 512], F32, tag="mm", name="mmg")
        nc.tensor.matmul(mmg[:], s1ds[:], glw_sb[:], start=True, stop=True)
        gpart = work.tile([1, 512], F32, tag="gpart", bufs=1)
        nc.vector.scalar_tensor_tensor(gpart[:], negs2w_sb[:], smur[:], mmg[:],
                                       OP.mult, OP.add)
        nc.sync.dma_start(ar3_in[:], gpart[:])
        nc.gpsimd.collective_compute(
            "AllReduce", OP.add, replica_groups=[list(range(NC_))],
            ins=[ar3_in.opt()], outs=[ar3_out.opt()])
        ggT0 = work.tile([128, 4], F32, tag="ggT0", bufs=1)
        nc.sync.dma_start(ggT0[:], bass.AP(ar3_out.tensor, 0, [[1, 128], [128, 4]]))
        nc.vector.tensor_tensor(ggT0[:], ggT0[:], grbT_sb[:], OP.add)
        ggT = work.tile([128, 4], BF16, tag="ggT", bufs=1)
        nc.scalar.activation(ggT[:], ggT0[:], AF.Gelu)
        attnF = work.tile([128, 8], F32, tag="attn", bufs=1)
        for j in range(8):
            mma = psum.tile([DL, 1], F32, tag="mm", name=f"mma{j}")
            for k in range(4):
                nc.tensor.matmul(mma[:], cswT_sb[k][:, 128 * j:128 * (j + 1)],
                                 ggT[:, k:k + 1], start=(k == 0), stop=(k == 3))
            nc.scalar.activation(attnF[:, j:j + 1], mma[:], AF.Sigmoid,
                                 bias=csbF_sb[:, j:j + 1])

        # ================= stage 6: local full-d output GEMM ================
        for j in range(8):
            nc.vector.tensor_scalar_mul(wcombF[j][:], wcombF[j][:], attnF[:, j:j + 1])
        wscF = wcombF
        for h in range(2):
            mmo = psum.tile([LC, DM], F32, tag="mm", name=f"mmo{h}")
            for p in range(8):
                nc.tensor.matmul(mmo[:], rT[2 * p + h][:], wscF[p][:],
                                 start=(p == 0), stop=(p == 7))
            ob = work.tile([LC, DM], BF16, tag="ob", bufs=2, name=f"ob{h}")
            if h == 0:
                nc.vector.tensor_copy(ob[:], mmo[:])
            else:
                nc.scalar.copy(ob[:], mmo[:])
            (nc.sync if h == 0 else nc.scalar).dma_start(
                out_shard[LC * h:LC * (h + 1), :], ob[:])

    nc.compile()
    return nc


# ---------------------------------------------------------------- host ---

def _prep_inputs(inputs):
    import ml_dtypes
    f = lambda k: np.ascontiguousarray(np.asarray(inputs[k], dtype=np.float32))
    hid = f('hidden_states')[0]
    hidT = np.ascontiguousarray(hid.T)
    in_proj = f('in_proj_w')
    scans = [
        ('conv1d_w', 'conv1d_bias', 'x_proj_w', 'dt_proj_w', 'dt_bias', 'A_log', 'D'),
        ('conv1d_b_w', 'conv1d_b_bias', 'x_proj_b_w', 'dt_proj_b_w', 'dt_b_bias', 'A_b_log', 'D_b'),
        ('conv1d_c_w', 'conv1d_c_bias', 'x_proj_c_w', 'dt_proj_c_w', 'dt_c_bias', 'A_c_log', 'D_c'),
    ]
    ln_g = f('att_ln_g'); ln_b = f('att_ln_b')
    gr_w = f('att_gr_w'); cs_w = f('att_cs_w')
    ow = f('att_out_w'); opw = f('out_proj_w')

    maps = []
    for c in range(NC_):
        dsl = slice(c * DL, (c + 1) * DL)
        m = dict(
            hidT=hidT.astype(ml_dtypes.bfloat16),
            wxT=np.ascontiguousarray(in_proj[c * DL:(c + 1) * DL, :].T).astype(ml_dtypes.bfloat16),
            wzT=np.ascontiguousarray(in_proj[DI + c * DL:DI + (c + 1) * DL, :].T).astype(ml_dtypes.bfloat16),
            ones_colf=np.ones((128, 1), np.float32),
            identT=np.eye(128, dtype=ml_dtypes.bfloat16),
            glw=np.ascontiguousarray((ln_g[dsl] / L)[:, None] * gr_w[:, dsl].T).astype(ml_dtypes.bfloat16),
            negs2w=np.ascontiguousarray(
                -((ln_g[dsl] / L)[:, None] * gr_w[:, dsl].T).sum(0).reshape(1, 512)),
            grbT=np.ascontiguousarray(
                (f('att_gr_b') + gr_w @ ln_b).reshape(4, 128).T),
            cswT=np.ascontiguousarray(cs_w.T).astype(ml_dtypes.bfloat16),
            csbF=np.ascontiguousarray(f('att_cs_b').reshape(8, 128).T),
            wcombF=np.ascontiguousarray(ow.T @ opw.T).astype(ml_dtypes.bfloat16),
        )
        for s, keys in enumerate(scans):
            cw, cb, xw, dtw, dtbk, alog, dk = keys
            cwd = np.zeros((4 * DL, DL), np.float32)
            for k in range(4):
                cwd[128 * k:128 * (k + 1), :][np.arange(DL), np.arange(DL)] = f(cw)[dsl, 0, k]
            m[f'convw{s}'] = cwd.astype(ml_dtypes.bfloat16)
            m[f'convb{s}'] = np.ascontiguousarray(f(cb)[dsl].reshape(DL, 1))
            m[f'xwT{s}'] = np.ascontiguousarray(f(xw)[:, dsl].T).astype(ml_dtypes.bfloat16)
            m[f'dtwT{s}'] = np.ascontiguousarray(f(dtw)[dsl, :].T).astype(ml_dtypes.bfloat16)
            m[f'dtb{s}'] = np.ascontiguousarray(f(dtbk)[dsl].reshape(DL, 1))
            m[f'avec{s}'] = np.ascontiguousarray(-np.exp(f(alog)[dsl]))
            dd = np.zeros((DL, DL), np.float32)
            dd[np.arange(DL), np.arange(DL)] = f(dk)[dsl]
            m[f'ddiag{s}'] = dd.astype(ml_dtypes.bfloat16)
        maps.append(m)
    bias_out = f('att_out_b') @ opw.T
    return maps, bias_out


def kernel(**inputs) -> np.ndarray:
    if 'nc' not in _CACHE:
        _CACHE['nc'] = _build(dbg=bool(os.environ.get('BASS_KERNEL_DBG')))
    nc = _CACHE['nc']
    maps, bias_out = _prep_inputs(inputs)
    trace = bool(os.environ.get('BASS_KERNEL_TRACE'))
    res = run_bass_kernel_spmd(nc, maps, list(range(NC_)), trace=trace)
    _CACHE['last_exec_ns'] = res.exec_time_ns
    _CACHE['last_res'] = res
    shards = [np.asarray(res.results[c]['out_shard']).astype(np.float32) for c in range(NC_)]
    out = np.concatenate(shards, axis=0) + bias_out[None, :]
    return out[None].astype(np.float32)
